# revision 1
# baseline (speedup 1.0000x reference)
"""CSPN (convolutional spatial propagation network) Trainium2 Bass kernel.

Problem: 16 iterations of
    d <- blend(max_c[ box3x3(g_c * d) / box3x3(g_c) ], sparse)
with g = |guidance| [B=8, C=8, H=256, W=512], d = depth [B,1,H,W].

Strategy (8 NeuronCores, pure batch data-parallel — one image per core):
- H=256 split into two partition blocks of 128 (hb0 = rows 0..127 stored
  ROW-REVERSED via a host-side flip, hb1 = rows 128..255 natural). With this
  layout both inter-block boundary rows live on partition 0, so the
  cross-block boundary fix-up is uniform.
- box3x3(e) is computed on the tensor engine: for each (channel, block) bank,
  three PSUM-accumulated matmuls with a tridiagonal ones stationary T and the
  rhs access pattern shifted by -1/0/+1 along W (zero-padded columns), plus
  ONE rank-3 patch matmul adding rowsum3 of the other block's boundary row
  (the three shifted copies of both boundary rows are gathered each iteration
  into a [3, 2, C, W] staging tile by a single SBUF-to-SBUF DMA).
- PSUM (fp32) is evacuated by the scalar engine (ACT) as fp16; the vector
  engine applies the precomputed (1/wsum)*(1-mask) factor, reduces the max
  over the 8 gate channels with a tensor_tensor max tree, and adds back the
  sparse depths (sparse is already mask*values, so blend = dmax + sparse).
- Everything in the loop is fp16 (DVE 2x mode) with fp32 PSUM accumulation.
"""

import sys
import numpy as np

sys.path.insert(0, "/opt/trn_rl_repo")

B, C, H, W = 8, 8, 256, 512
PB = 128          # partitions per h-block
WP = W + 2        # padded width (zero col at each end)
NUM_ITERS = 16
N_CORES = 8

_BUILT = None


def _bcast_c(ap, nch, bass):
    """Insert a broadcast (step 0) channel dim after the partition dim."""
    aps = [list(x) for x in ap.ap]
    assert len(aps) == 2
    return bass.AP(tensor=ap.tensor, offset=ap.offset,
                   ap=[aps[0], [0, nch], aps[1]])


def _build(num_iters=NUM_ITERS, variant="full"):
    import concourse.bacc as bacc
    import concourse.bass as bass
    import concourse.tile as tile
    from concourse import mybir

    f16, f32 = mybir.dt.float16, mybir.dt.float32
    Alu = mybir.AluOpType

    nc = bacc.Bacc("TRN2", target_bir_lowering=False, debug=False)

    g_d = nc.dram_tensor("guidance", [C, 2, PB, W], f32, kind="ExternalInput")
    b_d = nc.dram_tensor("blur", [2, PB, W], f32, kind="ExternalInput")
    s_d = nc.dram_tensor("sparse", [2, PB, W], f32, kind="ExternalInput")
    t_d = nc.dram_tensor("tband", [PB, PB], f16, kind="ExternalInput")
    o_d = nc.dram_tensor("out", [2, PB, W], f32, kind="ExternalOutput")

    with tile.TileContext(nc) as tc, nc.allow_low_precision(
        "fp16 by design: diffusion is a convex combination, error stays ~eps"
    ):
        import contextlib
        ctx = contextlib.ExitStack()
        with ctx:
            perm = ctx.enter_context(tc.tile_pool(name="perm", bufs=1))
            epool = ctx.enter_context(tc.tile_pool(name="ep", bufs=2))
            g_t = perm.tile([PB, 2, C, WP], f16)
            ivm_t = perm.tile([PB, 2, C, W], f16)  # (1/wsum)*(1-mask)
            sraw_t = perm.tile([PB, 2, C, W], f16)
            sc_t = perm.tile([PB, 2, C, W], f16)
            l1_t = perm.tile([PB, 2, 4, W], f16)
            l2_t = perm.tile([PB, 2, 2, W], f16)
            dmax_t = perm.tile([PB, 2, W], f16)
            d_t = perm.tile([PB, 2, WP], f16)
            sp_t = perm.tile([PB, 2, W], f16)
            o32_t = perm.tile([PB, 2, W], f32)
            T_t = perm.tile([PB, PB], f16)

            nc.sync.dma_start(T_t[:], t_d[:])

            psum = ctx.enter_context(
                tc.tile_pool(name="ps", bufs=8, space="PSUM"))

            HC = C // 2  # channels per pipeline unit

            def box_mains(hb, h, src):
                """3 shifted tridiagonal matmuls per channel bank, + evac.
                src: [PB, HC, WP] padded tile for unit (hb, h)."""
                tiles = [psum.tile([PB, W], f32, tag="ps", name=f"ps{c}")
                         for c in range(HC)]
                for c in range(HC):
                    for s in range(3):
                        nc.tensor.matmul(tiles[c][:], T_t[:],
                                         src[:, c, s:s + W],
                                         start=(s == 0), stop=(s == 2))
                    nc.scalar.copy(sraw_t[:, hb, h * HC + c], tiles[c][:])

            def bnd_stage(hb, h, src):
                """Stage unit (hb,h)'s boundary row one-channel-per-partition
                and rowsum3 it: bs[c, w] = sum_s src[part0, c, w+s]."""
                bt = epool.tile([HC, WP], f16, tag=f"bt{hb}{h}",
                                name=f"bt{hb}{h}")
                bs = epool.tile([HC, W], f16, tag=f"bs{hb}{h}",
                                name=f"bs{hb}{h}")
                nc.sync.dma_start(bt[:], src[0:1, :, :])
                nc.vector.tensor_add(bs[:], bt[:, 0:W], bt[:, 2:2 + W])
                nc.vector.tensor_add(bs[:], bs[:], bt[:, 1:1 + W])
                return bs

            def bnd_add(hb, h, bs):
                """sraw[row0 of unit] += staged rowsum3 (one CCE-add DMA)."""
                nc.gpsimd.dma_start(sraw_t[0:1, hb, h * HC:(h + 1) * HC, :],
                                    bs[:], accum_op=Alu.add)

            def back_half(hb, h):
                """scale by ivm and 4-way max within the unit's channels."""
                c0 = h * HC
                nc.vector.tensor_mul(sc_t[:, hb, c0:c0 + HC],
                                     sraw_t[:, hb, c0:c0 + HC],
                                     ivm_t[:, hb, c0:c0 + HC])
                r = sc_t[:, hb, c0:c0 + HC].rearrange(
                    "p (x y) w -> p x y w", y=2)
                nc.vector.tensor_max(l1_t[:, hb, 2 * h:2 * h + 2],
                                     r[:, :, 0], r[:, :, 1])
                nc.vector.tensor_max(l2_t[:, hb, h],
                                     l1_t[:, hb, 2 * h], l1_t[:, hb, 2 * h + 1])

            def back_combine(hb):
                nc.vector.tensor_max(dmax_t[:, hb],
                                     l2_t[:, hb, 0], l2_t[:, hb, 1])
                nc.vector.tensor_add(d_t[:, hb, 1:1 + W],
                                     dmax_t[:, hb], sp_t[:, hb])

            def front_half(hb, h):
                """e = g * d for 4 channels; stage its boundary row."""
                e = epool.tile([PB, HC, WP], f16, tag=f"e{hb}{h}",
                               name=f"e{hb}{h}")
                dbc = _bcast_c(d_t[:, hb], HC, bass)
                nc.vector.tensor_mul(e[:], g_t[:, hb, h * HC:(h + 1) * HC],
                                     dbc)
                return e, bnd_stage(hb, h, e)

            # ---------------- pre-loop ----------------
            with tc.tile_pool(name="trans", bufs=1) as trans:
                b32 = trans.tile([PB, 2, W], f32, tag="b32")
                s32 = trans.tile([PB, 2, W], f32, tag="s32")
                m_t = trans.tile([PB, 2, W], f16, tag="m16")
                im_t = trans.tile([PB, 2, W], f16, tag="im16")
                b16 = trans.tile([PB, 2, W], f16, tag="b16")
                tmp_t = trans.tile([PB, 2, W], f16, tag="t16")

                nc.sync.dma_start(b32[:], b_d[:].rearrange("h p w -> p h w"))
                nc.sync.dma_start(s32[:], s_d[:].rearrange("h p w -> p h w"))

                nc.vector.tensor_copy(sp_t[:], s32[:])
                nc.scalar.sign(m_t[:], s32[:])  # sparse>=0 -> mask in {0,1}
                nc.vector.tensor_scalar(im_t[:], m_t[:], -1.0, 1.0,
                                        Alu.mult, Alu.add)
                nc.vector.tensor_copy(b16[:], b32[:])
                nc.vector.tensor_mul(tmp_t[:], im_t[:], b16[:])
                nc.vector.memset(d_t[:], 0.0)
                nc.vector.tensor_add(d_t[:, :, 1:1 + W], tmp_t[:], sp_t[:])

                # guidance |.| -> fp16 padded layout
                for hb in range(2):
                    gf = trans.tile([PB, C, W], f32, tag="gf32")
                    nc.sync.dma_start(
                        gf[:], g_d[:, hb].rearrange("c p w -> p c w"))
                    nc.vector.memset(g_t[:, hb, :, 0:1], 0.0)
                    nc.vector.memset(g_t[:, hb, :, WP - 1:WP], 0.0)
                    nc.scalar.activation(g_t[:, hb, :, 1:1 + W], gf[:],
                                         mybir.ActivationFunctionType.Abs)

                # wsum -> ivm = (1/wsum)*(1-mask)
                gbs = {(hb, h): bnd_stage(hb, h,
                                          g_t[:, hb, h * HC:(h + 1) * HC])
                       for hb in range(2) for h in range(2)}
                for hb in range(2):
                    for h in range(2):
                        box_mains(hb, h, g_t[:, hb, h * HC:(h + 1) * HC])
                        bnd_add(hb, h, gbs[(1 - hb, h)])
                for hb in range(2):
                    w32 = trans.tile([PB, C, W], f32, tag="w32")
                    iw32 = trans.tile([PB, C, W], f32, tag="iw32")
                    nc.vector.tensor_copy(w32[:], sraw_t[:, hb])
                    nc.vector.reciprocal_approx_fast(out=iw32[:], in_=w32[:])
                    nc.vector.tensor_copy(sc_t[:, hb], iw32[:])  # f32->f16
                    imb = _bcast_c(im_t[:, hb], C, bass)
                    nc.vector.tensor_mul(ivm_t[:, hb], sc_t[:, hb], imb)

            # ------- 16 diffusion iterations, software-pipelined -------
            # 4 pipeline units per iteration: (hb, channel-half)
            bss = {}
            for hb in (0, 1):
                for h in (0, 1):
                    e, bs = front_half(hb, h)
                    bss[(hb, h)] = bs
                    box_mains(hb, h, e)
            do_pe = variant in ("full", "pe")
            do_back = variant in ("full", "dve")
            for t in range(num_iters):
                last = t == num_iters - 1
                if do_pe:
                    for hb in (0, 1):
                        for h in (0, 1):
                            bnd_add(hb, h, bss[(1 - hb, h)])
                newbs = {}
                for hb in (0, 1):
                    if do_back:
                        back_half(hb, 0)
                        back_half(hb, 1)
                        back_combine(hb)
                    if not last:
                        for h in (0, 1):
                            e, bs = front_half(hb, h)
                            newbs[(hb, h)] = bs
                            if do_pe:
                                box_mains(hb, h, e)
                bss.update(newbs)

            # ---------------- output ----------------
            nc.vector.tensor_copy(o32_t[:], d_t[:, :, 1:1 + W])
            nc.sync.dma_start(o_d[:].rearrange("h p w -> p h w"), o32_t[:])

    nc.compile()
    return nc


def _get_built():
    global _BUILT
    if _BUILT is None:
        _BUILT = _build()
    return _BUILT


def _host_prep(guidance, blur_depth, sparse_depth):
    """Shard batch across cores; flip rows 0..127 so hb0 is row-reversed."""
    tband = np.zeros((PB, PB), np.float16)
    for k in range(PB):
        for p in range(max(0, k - 1), min(PB, k + 2)):
            tband[k, p] = 1.0
    in_maps = []
    for b in range(guidance.shape[0]):
        g = guidance[b].astype(np.float32, copy=False)
        bl = blur_depth[b, 0].astype(np.float32, copy=False)
        sp = sparse_depth[b, 0].astype(np.float32, copy=False)
        gp = np.ascontiguousarray(
            np.stack([g[:, 127::-1, :], g[:, 128:, :]], axis=1))
        bp = np.ascontiguousarray(np.stack([bl[127::-1, :], bl[128:, :]]))
        spp = np.ascontiguousarray(np.stack([sp[127::-1, :], sp[128:, :]]))
        in_maps.append({
            "guidance": gp, "blur": bp, "sparse": spp, "tband": tband,
        })
    return in_maps


def _host_post(results):
    n = len(results)
    out = np.empty((n, 1, H, W), np.float32)
    for b in range(n):
        o = results[b]["out"]  # [2, 128, 512]
        out[b, 0, 0:PB] = o[0, ::-1, :]
        out[b, 0, PB:] = o[1]
    return out


def kernel(guidance, blur_depth, sparse_depth):
    from concourse.bass_utils import run_bass_kernel_spmd

    nc = _get_built()
    in_maps = _host_prep(guidance, blur_depth, sparse_depth)
    res = run_bass_kernel_spmd(nc, in_maps, core_ids=list(range(N_CORES)))
    return _host_post(res.results)


if __name__ == "__main__":
    rng = np.random.default_rng(0)
    g = np.abs(rng.standard_normal((B, C, H, W), dtype=np.float32))
    bl = rng.random((B, 1, H, W), dtype=np.float32)
    sp = rng.random((B, 1, H, W), dtype=np.float32)
    sp *= (rng.random((B, 1, H, W)) < 0.05)
    out = kernel(g, bl, sp)
    print(out.shape, out.dtype, np.isfinite(out).all())



# revision 8
# speedup vs baseline: 51.3645x; 51.3645x over previous
"""CSPN (convolutional spatial propagation network) Trainium2 Bass kernel.

Problem: 16 iterations of
    d <- blend(max_c[ box3x3(g_c * d) / box3x3(g_c) ], sparse)
with g = |guidance| [B=8, C=8, H=256, W=512], d = depth [B,1,H,W].

Strategy (8 NeuronCores, pure batch data-parallel — one image per core):
- H=256 split into two partition blocks of 128 (hb0 = rows 0..127 stored
  ROW-REVERSED via a host-side flip, hb1 = rows 128..255 natural). With this
  layout both inter-block boundary rows live on partition 0, so the
  cross-block boundary fix-up is uniform.
- box3x3(e) is computed on the tensor engine: for each (channel, block) bank,
  three PSUM-accumulated matmuls with a tridiagonal ones stationary T and the
  rhs access pattern shifted by -1/0/+1 along W (zero-padded columns), plus
  ONE rank-3 patch matmul adding rowsum3 of the other block's boundary row
  (the three shifted copies of both boundary rows are gathered each iteration
  into a [3, 2, C, W] staging tile by a single SBUF-to-SBUF DMA).
- PSUM (fp32) is evacuated by the scalar engine (ACT) as fp16; the vector
  engine applies the precomputed (1/wsum)*(1-mask) factor, reduces the max
  over the 8 gate channels with a tensor_tensor max tree, and adds back the
  sparse depths (sparse is already mask*values, so blend = dmax + sparse).
- Everything in the loop is fp16 (DVE 2x mode) with fp32 PSUM accumulation.
"""

import sys
import numpy as np

sys.path.insert(0, "/opt/trn_rl_repo")

B, C, H, W = 8, 8, 256, 512
PB = 128          # partitions per h-block
WP = W + 2        # padded width (zero col at each end)
NUM_ITERS = 16
N_CORES = 8

_BUILT = None


def _bcast_c(ap, nch, bass):
    """Insert a broadcast (step 0) channel dim after the partition dim."""
    aps = [list(x) for x in ap.ap]
    assert len(aps) == 2
    return bass.AP(tensor=ap.tensor, offset=ap.offset,
                   ap=[aps[0], [0, nch], aps[1]])


def _build(num_iters=NUM_ITERS, variant="full", outer_loops=1):
    import concourse.bacc as bacc
    import concourse.bass as bass
    import concourse.tile as tile
    from concourse import mybir

    f16, f32 = mybir.dt.float16, mybir.dt.float32
    Alu = mybir.AluOpType

    nc = bacc.Bacc("TRN2", target_bir_lowering=False, debug=False)

    g_d = nc.dram_tensor("guidance", [C, 2, PB, W], f32, kind="ExternalInput")
    b_d = nc.dram_tensor("blur", [2, PB, W], f32, kind="ExternalInput")
    s_d = nc.dram_tensor("sparse", [2, PB, W], f32, kind="ExternalInput")
    t_d = nc.dram_tensor("tband", [PB, PB], f16, kind="ExternalInput")
    o_d = nc.dram_tensor("out", [2, PB, W], f32, kind="ExternalOutput")

    with tile.TileContext(nc) as tc, nc.allow_low_precision(
        "fp16 by design: diffusion is a convex combination, error stays ~eps"
    ):
        import contextlib
        ctx = contextlib.ExitStack()
        with ctx:
            perm = ctx.enter_context(tc.tile_pool(name="perm", bufs=1))
            epool = ctx.enter_context(tc.tile_pool(name="ep", bufs=2))
            g_t = perm.tile([PB, 2, C, WP], f16)
            ivm_t = perm.tile([PB, 2, C, W], f16)  # (1/wsum)*(1-mask)
            sraw_t = perm.tile([PB, 2, C, W], f16)
            sc_t = perm.tile([PB, 2, C, W], f16)
            l1_t = perm.tile([PB, 2, 4, W], f16)
            l2_t = perm.tile([PB, 2, 2, W], f16)
            dmax_t = perm.tile([PB, 2, W], f16)
            d_t = perm.tile([PB, 2, WP], f16)
            sp_t = perm.tile([PB, 2, W], f16)
            o32_t = perm.tile([PB, 2, W], f32)
            T_t = perm.tile([PB, PB], f16)

            nc.sync.dma_start(T_t[:], t_d[:])

            psum = ctx.enter_context(
                tc.tile_pool(name="ps", bufs=8, space="PSUM"))

            HC = C // 2  # channels per pipeline unit

            def box_mains(hb, h, src):
                """3 shifted tridiagonal matmuls per channel bank, + evac.
                src: [PB, HC, WP] padded tile for unit (hb, h)."""
                tiles = [psum.tile([PB, W], f32, tag="ps", name=f"ps{c}")
                         for c in range(HC)]
                for c in range(HC):
                    for s in range(3):
                        nc.tensor.matmul(tiles[c][:], T_t[:],
                                         src[:, c, s:s + W],
                                         start=(s == 0), stop=(s == 2))
                    nc.scalar.copy(sraw_t[:, hb, h * HC + c], tiles[c][:])

            def bnd_stage(hb, h, src):
                """Stage unit (hb,h)'s boundary row one-channel-per-partition
                and rowsum3 it: bs[c, w] = sum_s src[part0, c, w+s]."""
                bt = epool.tile([HC, WP], f16, tag=f"bt{hb}{h}",
                                name=f"bt{hb}{h}")
                bs = epool.tile([HC, W], f16, tag=f"bs{hb}{h}",
                                name=f"bs{hb}{h}")
                nc.sync.dma_start(bt[:], src[0:1, :, :])
                nc.vector.tensor_add(bs[:], bt[:, 0:W], bt[:, 2:2 + W])
                nc.vector.tensor_add(bs[:], bs[:], bt[:, 1:1 + W])
                return bs

            def bnd_add(hb, h, bs):
                """sraw[row0 of unit] += staged rowsum3 (one CCE-add DMA)."""
                nc.gpsimd.dma_start(sraw_t[0:1, hb, h * HC:(h + 1) * HC, :],
                                    bs[:], accum_op=Alu.add)

            def back_half(hb, h):
                """scale by ivm and 4-way max within the unit's channels."""
                c0 = h * HC
                nc.vector.tensor_mul(sc_t[:, hb, c0:c0 + HC],
                                     sraw_t[:, hb, c0:c0 + HC],
                                     ivm_t[:, hb, c0:c0 + HC])
                r = sc_t[:, hb, c0:c0 + HC].rearrange(
                    "p (x y) w -> p x y w", y=2)
                nc.vector.tensor_max(l1_t[:, hb, 2 * h:2 * h + 2],
                                     r[:, :, 0], r[:, :, 1])
                nc.vector.tensor_max(l2_t[:, hb, h],
                                     l1_t[:, hb, 2 * h], l1_t[:, hb, 2 * h + 1])

            def back_combine(hb):
                nc.vector.tensor_max(dmax_t[:, hb],
                                     l2_t[:, hb, 0], l2_t[:, hb, 1])
                nc.vector.tensor_add(d_t[:, hb, 1:1 + W],
                                     dmax_t[:, hb], sp_t[:, hb])

            def front_half(hb, h):
                """e = g * d for 4 channels; stage its boundary row."""
                e = epool.tile([PB, HC, WP], f16, tag=f"e{hb}{h}",
                               name=f"e{hb}{h}")
                dbc = _bcast_c(d_t[:, hb], HC, bass)
                nc.vector.tensor_mul(e[:], g_t[:, hb, h * HC:(h + 1) * HC],
                                     dbc)
                return e, bnd_stage(hb, h, e)

            # ---------------- pre-loop ----------------
            with tc.tile_pool(name="trans", bufs=1) as trans:
                b32 = trans.tile([PB, 2, W], f32, tag="b32")
                s32 = trans.tile([PB, 2, W], f32, tag="s32")
                m_t = trans.tile([PB, 2, W], f16, tag="m16")
                im_t = trans.tile([PB, 2, W], f16, tag="im16")
                b16 = trans.tile([PB, 2, W], f16, tag="b16")
                tmp_t = trans.tile([PB, 2, W], f16, tag="t16")

                nc.sync.dma_start(b32[:], b_d[:].rearrange("h p w -> p h w"))
                nc.sync.dma_start(s32[:], s_d[:].rearrange("h p w -> p h w"))

                nc.vector.tensor_copy(sp_t[:], s32[:])
                nc.scalar.sign(m_t[:], s32[:])  # sparse>=0 -> mask in {0,1}
                nc.vector.tensor_scalar(im_t[:], m_t[:], -1.0, 1.0,
                                        Alu.mult, Alu.add)
                nc.vector.tensor_copy(b16[:], b32[:])
                nc.vector.tensor_mul(tmp_t[:], im_t[:], b16[:])
                nc.vector.memset(d_t[:], 0.0)
                nc.vector.tensor_add(d_t[:, :, 1:1 + W], tmp_t[:], sp_t[:])

                # guidance |.| -> fp16 padded layout
                for hb in range(2):
                    gf = trans.tile([PB, C, W], f32, tag="gf32")
                    nc.sync.dma_start(
                        gf[:], g_d[:, hb].rearrange("c p w -> p c w"))
                    nc.vector.memset(g_t[:, hb, :, 0:1], 0.0)
                    nc.vector.memset(g_t[:, hb, :, WP - 1:WP], 0.0)
                    nc.scalar.activation(g_t[:, hb, :, 1:1 + W], gf[:],
                                         mybir.ActivationFunctionType.Abs)

                # wsum -> ivm = (1/wsum)*(1-mask)
                gbs = {(hb, h): bnd_stage(hb, h,
                                          g_t[:, hb, h * HC:(h + 1) * HC])
                       for hb in range(2) for h in range(2)}
                for hb in range(2):
                    for h in range(2):
                        box_mains(hb, h, g_t[:, hb, h * HC:(h + 1) * HC])
                        bnd_add(hb, h, gbs[(1 - hb, h)])
                for hb in range(2):
                    w32 = trans.tile([PB, C, W], f32, tag="w32")
                    iw32 = trans.tile([PB, C, W], f32, tag="iw32")
                    nc.vector.tensor_copy(w32[:], sraw_t[:, hb])
                    nc.vector.reciprocal_approx_fast(out=iw32[:], in_=w32[:])
                    nc.vector.tensor_copy(sc_t[:, hb], iw32[:])  # f32->f16
                    imb = _bcast_c(im_t[:, hb], C, bass)
                    nc.vector.tensor_mul(ivm_t[:, hb], sc_t[:, hb], imb)

            # ------- 16 diffusion iterations, software-pipelined -------
            # 4 pipeline units per iteration: (hb, channel-half)
            bss = {}
            for hb in (0, 1):
                for h in (0, 1):
                    e, bs = front_half(hb, h)
                    bss[(hb, h)] = bs
                    box_mains(hb, h, e)
            do_pe = variant in ("full", "pe")
            do_back = variant in ("full", "dve")

            def loop_body(steady_state=False):
                for t in range(num_iters):
                    last = (t == num_iters - 1) and not steady_state
                    if do_pe:
                        for hb in (0, 1):
                            for h in (0, 1):
                                bnd_add(hb, h, bss[(1 - hb, h)])
                    newbs = {}
                    for hb in (0, 1):
                        if do_back:
                            back_half(hb, 0)
                            back_half(hb, 1)
                            back_combine(hb)
                        if not last:
                            for h in (0, 1):
                                e, bs = front_half(hb, h)
                                newbs[(hb, h)] = bs
                                if do_pe:
                                    box_mains(hb, h, e)
                    bss.update(newbs)

            if outer_loops == 1:
                loop_body()
            else:
                # timing-only: repeat a steady-state body on a hardware loop
                with tc.For_i(0, outer_loops, 1):
                    loop_body(steady_state=True)

            # ---------------- output ----------------
            nc.vector.tensor_copy(o32_t[:], d_t[:, :, 1:1 + W])
            nc.sync.dma_start(o_d[:].rearrange("h p w -> p h w"), o32_t[:])

    nc.compile()
    return nc


def _get_built():
    global _BUILT
    if _BUILT is None:
        _BUILT = _build()
    return _BUILT


def _host_prep(guidance, blur_depth, sparse_depth):
    """Shard batch across cores; flip rows 0..127 so hb0 is row-reversed."""
    tband = np.zeros((PB, PB), np.float16)
    for k in range(PB):
        for p in range(max(0, k - 1), min(PB, k + 2)):
            tband[k, p] = 1.0
    in_maps = []
    for b in range(guidance.shape[0]):
        g = guidance[b].astype(np.float32, copy=False)
        bl = blur_depth[b, 0].astype(np.float32, copy=False)
        sp = sparse_depth[b, 0].astype(np.float32, copy=False)
        gp = np.ascontiguousarray(
            np.stack([g[:, 127::-1, :], g[:, 128:, :]], axis=1))
        bp = np.ascontiguousarray(np.stack([bl[127::-1, :], bl[128:, :]]))
        spp = np.ascontiguousarray(np.stack([sp[127::-1, :], sp[128:, :]]))
        in_maps.append({
            "guidance": gp, "blur": bp, "sparse": spp, "tband": tband,
        })
    return in_maps


def _host_post(results):
    n = len(results)
    out = np.empty((n, 1, H, W), np.float32)
    for b in range(n):
        o = results[b]["out"]  # [2, 128, 512]
        out[b, 0, 0:PB] = o[0, ::-1, :]
        out[b, 0, PB:] = o[1]
    return out


def kernel(guidance, blur_depth, sparse_depth):
    from concourse.bass_utils import run_bass_kernel_spmd

    nc = _get_built()
    in_maps = _host_prep(guidance, blur_depth, sparse_depth)
    res = run_bass_kernel_spmd(nc, in_maps, core_ids=list(range(N_CORES)))
    return _host_post(res.results)


if __name__ == "__main__":
    rng = np.random.default_rng(0)
    g = np.abs(rng.standard_normal((B, C, H, W), dtype=np.float32))
    bl = rng.random((B, 1, H, W), dtype=np.float32)
    sp = rng.random((B, 1, H, W), dtype=np.float32)
    sp *= (rng.random((B, 1, H, W)) < 0.05)
    out = kernel(g, bl, sp)
    print(out.shape, out.dtype, np.isfinite(out).all())

